# revision 20
# baseline (speedup 1.0000x reference)
"""NetVLAD Trainium2 Bass kernel, v3 (bf16 matmuls, fused slg, col-tiled acc).

Full inputs in, full output out. Data-parallel over batch N=64 across 8
NeuronCores (8 samples per core); conv weight and centroids replicated.

Structure (per core, 8 samples, 38 pixel-chunks of 128 per sample):
  - x arrives bf16 (host-converted): half the HBM traffic, and every PE
    stream runs at 1 cycle/row (fp32 would be 4).
  - Per chunk: transpose-mode matmul writes x_c^T to a *bf16* PSUM bank
    (8 chunks/bank), evacuated to SBUF by DVE in the 2x packed mode /
    ACT; a second matmul with the same stationary x_c produces logits
    into an f32 PSUM pool large enough for a whole sample (5 banks).
  - slg = logits * inv_s is computed by DVE directly from logits PSUM
    (per-bank ops, step-0 broadcast of inv_s), which both evacuates and
    scales in one 1x pass and releases the logits banks.
  - ss = sum_d x^2: ACT squares, GPSIMD does one pairwise fold (128->64),
    DVE reduces the rest. inv_s = exp(-0.5 ln ss) on ACT.
  - softmax: exp on ACT, Z-reduce + reciprocal on DVE, t = inv_s/Z and
    sbt = et * t broadcasts on GPSIMD.
  - acc matmuls are column-tiled: even chunks accumulate into PSUM
    partitions 0:64, odd chunks into 64:128, running pairwise-concurrent
    in the PE array; the two halves are summed during evacuation.
  - Tail (VLAD normalizations) uses gpsimd.partition_all_reduce for the
    global norm, so no PSUM bank is needed for tiny matmuls.
  PSUM budget: 2 (xt bf16) + 5 (logits f32) + 1 (acc) = 8 banks.
"""

import sys

if "/opt/trn_rl_repo" not in sys.path:
    sys.path.insert(0, "/opt/trn_rl_repo")

import numpy as np
from contextlib import ExitStack

N, D, HW, K = 64, 128, 4800, 64
NCORES = 8
NS = N // NCORES  # samples per core

CHUNKS = [(i * 128, min(128, HW - i * 128)) for i in range((HW + 127) // 128)]
NCH = len(CHUNKS)  # 38: 37 full + one 64-wide
XTS = 130  # xtr row stride (129 used; 130 keeps 4B alignment of chunk rows)

# waves of chunks sharing one xt PSUM bank / one lg PSUM bank
WAVES = [(w * 8, min(8, NCH - w * 8)) for w in range((NCH + 7) // 8)]  # 5 waves
NLG = len(WAVES)

PIPE = 3  # acc matmuls of sample n-PIPE run during round n

_CACHE = {}


def _patch_act_tables():
    """Steer bacc's ACT table-set placement so ln/exp (and the cheap
    square/copy fillers) live in one set: a single ACT_TABLE_LOAD."""
    if _CACHE.get("act_patched"):
        return
    from concourse import bacc, mybir

    orig = bacc.get_activation_tables
    AF = mybir.ActivationFunctionType
    combo = "natural_log_exp_and_others"

    def patched(arch):
        t = {k: set(v) for k, v in orig(arch).items()}
        if combo in t:
            for name in t:
                if name != combo:
                    t[name] = t[name] - {AF.Ln, AF.Exp}
        return t

    bacc.get_activation_tables = patched
    _CACHE["act_patched"] = True


def _build_nc():
    import concourse.tile as tile
    from concourse import bacc, mybir

    _patch_act_tables()

    nc = bacc.Bacc(
        "TRN2",
        target_bir_lowering=False,
        debug=False,
        enable_asserts=False,
        num_devices=NCORES,
    )
    x_ap = nc.dram_tensor(
        "x", [NS, D, HW], mybir.dt.bfloat16, kind="ExternalInput"
    ).ap()
    wt_ap = nc.dram_tensor("wt", [D, K], mybir.dt.bfloat16, kind="ExternalInput").ap()
    cent_ap = nc.dram_tensor(
        "cent", [K, D], mybir.dt.float32, kind="ExternalInput"
    ).ap()
    out_ap = nc.dram_tensor(
        "out", [NS, K, D], mybir.dt.float32, kind="ExternalOutput"
    ).ap()

    with tile.TileContext(nc) as tc:
        with ExitStack() as ctx:
            _body(ctx, tc, out_ap, x_ap, wt_ap, cent_ap)
    nc.compile()
    return nc


def _body(ctx, tc, out_ap, x_ap, wt_ap, cent_ap):
    import concourse.bass as bass
    from concourse import bass_isa, masks, mybir

    nc = tc.nc
    f32 = mybir.dt.float32
    bf16 = mybir.dt.bfloat16
    AF = mybir.ActivationFunctionType
    ALU = mybir.AluOpType
    X_AX = mybir.AxisListType.X

    singles = ctx.enter_context(tc.tile_pool(name="singles", bufs=1))
    xspool = ctx.enter_context(tc.tile_pool(name="xspool", bufs=3))
    xtrpool = ctx.enter_context(tc.tile_pool(name="xtrpool", bufs=PIPE + 1))
    x2pool = ctx.enter_context(tc.tile_pool(name="x2pool", bufs=2))
    slgpool = ctx.enter_context(tc.tile_pool(name="slgpool", bufs=2))
    etpool = ctx.enter_context(tc.tile_pool(name="etpool", bufs=2))
    sbtpool = ctx.enter_context(tc.tile_pool(name="sbtpool", bufs=PIPE))
    smalls = ctx.enter_context(tc.tile_pool(name="smalls", bufs=2))
    tails = ctx.enter_context(tc.tile_pool(name="tails", bufs=1))
    pp_xt = ctx.enter_context(tc.tile_pool(name="pp_xt", bufs=2, space="PSUM"))
    pp_lg = ctx.enter_context(tc.tile_pool(name="pp_lg", bufs=NLG, space="PSUM"))
    pp_acc = ctx.enter_context(tc.tile_pool(name="pp_acc", bufs=1, space="PSUM"))

    def bcast(ap, n):
        # append a step-0 free dim: [..., n] broadcast view
        return bass.AP(tensor=ap.tensor, offset=ap.offset, ap=list(ap.ap) + [[0, n]])

    def mid_bcast(ap, n):
        # [p, f] -> [p, n, f] with step-0 middle dim
        return bass.AP(
            tensor=ap.tensor,
            offset=ap.offset,
            ap=[ap.ap[0], [0, n]] + list(ap.ap[1:]),
        )

    # constants
    identb = singles.tile([128, 128], bf16)
    masks.make_identity(nc, identb[:])
    wtb = singles.tile([D, K], bf16)
    nc.sync.dma_start(out=wtb[:], in_=wt_ap[:])
    cent_s = singles.tile([K, D], f32)
    nc.sync.dma_start(out=cent_s[:], in_=cent_ap[:])

    state = {}  # n -> dict of live tiles
    cstate = {}  # n -> acc psum tile

    def emit_load(n):
        xs = xspool.tile([D, HW], bf16, tag="xs", name="xs")
        nc.sync.dma_start(out=xs[:, 0 : HW // 2], in_=x_ap[n, :, 0 : HW // 2])
        nc.sync.dma_start(out=xs[:, HW // 2 :], in_=x_ap[n, :, HW // 2 :])
        return xs

    # ---- softmax part A of sample m, split into pieces interleaved with
    # the next round's waves (slg / exp / zz / recip / tsc) ----
    def sm_slg(m, tiles):
        st = state[m]
        is_ = st["is_"]
        if "slg" not in st:
            st["slg"] = slgpool.tile([128, NCH, K], bf16, tag="slg", name="slg")
            st["et"] = etpool.tile([128, NCH, K], bf16, tag="et", name="et")
            st["zz"] = smalls.tile([128, NCH], f32, tag="zz", name="zz")
        slg = st["slg"]
        # slg = lg * inv_s straight out of PSUM (evac + scale in one pass);
        # releases lg bank t for the next round's logits matmuls
        for t in tiles:
            lg_p = st["lgp"][t]
            c0, wn = WAVES[t]
            nc.vector.tensor_tensor(
                out=slg[:, c0 : c0 + wn, :],
                in0=lg_p[:, 0:wn, :],
                in1=bcast(is_[:, c0 : c0 + wn], K),
                op=ALU.mult,
            )

    SMH = 2 * (NCH // 4)  # 18

    def sm_exp(m, h):
        st = state[m]
        h0, h1 = (0, SMH) if h == 0 else (SMH, NCH)
        nc.scalar.activation(
            st["et"][:, h0:h1, :], st["slg"][:, h0:h1, :], AF.Exp
        )

    def sm_zz(m, h):
        st = state[m]
        h0, h1 = (0, SMH) if h == 0 else (SMH, NCH)
        nc.vector.tensor_reduce(
            out=st["zz"][:, h0:h1], in_=st["et"][:, h0:h1, :], axis=X_AX, op=ALU.add
        )

    def sm_tsc(m):
        st = state[m]
        rr = smalls.tile([128, NCH], f32, tag="rr", name="rr")
        tsc = smalls.tile([128, NCH], bf16, tag="tsc", name="tsc")
        st["tsc"] = tsc
        nc.vector.reciprocal(rr[:], st["zz"][:])
        nc.gpsimd.tensor_tensor(out=tsc[:], in0=st["is_"], in1=rr[:], op=ALU.mult)

    # ---- softmax part B of sample m (runs at round m+2 start: ready work
    # for GPSIMD while everything else in the round is still blocked) ----
    def sm_sbt(m):
        st = state[m]
        et, tsc = st["et"], st["tsc"]
        sbt = sbtpool.tile([128, NCH, K], bf16, tag="sbt", name="sbt")
        st["sbt"] = sbt
        for h0, h1 in ((0, SMH), (SMH, NCH)):
            nc.gpsimd.tensor_tensor(
                out=sbt[:, h0:h1, :],
                in0=et[:, h0:h1, :],
                in1=bcast(tsc[:, h0:h1], K),
                op=ALU.mult,
            )

    # ---- transpose waves of sample n ----
    def emit_wave(n, w):
        st = state[n]
        xs, xtr = st["xs"], st["xtr"]
        c0, wn = WAVES[w]
        xt_p = pp_xt.tile([128, 8, 128], bf16, tag="xt", name="xt")
        for j in range(wn):
            p0, cw = CHUNKS[c0 + j]
            nc.tensor.transpose(xt_p[:cw, j, :], xs[:, p0 : p0 + cw], identb[:])
        dst = xtr[:, c0 : c0 + wn, 0:128]
        src = xt_p[:, 0:wn, :]
        if w % 2 == 0:
            nc.vector.tensor_copy(dst, src)
        else:
            nc.scalar.copy(dst, src)

    def emit_lg_tile(n, t):
        st = state[n]
        xs = st["xs"]
        if "lgp" not in st:
            st["lgp"] = {}
        c0, wn = WAVES[t]
        lg_p = pp_lg.tile([128, 8, K], f32, tag="lg", name="lg")
        for j in range(wn):
            p0, cw = CHUNKS[c0 + j]
            nc.tensor.matmul(
                lg_p[:cw, j, :],
                lhsT=xs[:, p0 : p0 + cw],
                rhs=wtb[:],
                start=True,
                stop=True,
            )
        st["lgp"][t] = lg_p

    # ---- ss chain of sample n (round tail): is_ ready for round n+1 ----
    SSH = NCH // 2  # 19

    def ss_alloc(n):
        st = state[n]
        st["x2"] = x2pool.tile([128, NCH, 128], bf16, tag="x2", name="x2")
        st["t64"] = x2pool.tile([128, NCH, 64], bf16, tag="t64", name="t64")
        st["t32"] = x2pool.tile([128, NCH, 32], bf16, tag="t32", name="t32")
        st["ss"] = smalls.tile([128, NCH], f32, tag="ss", name="ss")

    def ss_sq(n, h):
        st = state[n]
        h0, h1 = (0, SSH) if h == 0 else (SSH, NCH)
        nc.scalar.activation(
            st["x2"][:, h0:h1, :], st["xtr"][:, h0:h1, 0:128], AF.Square
        )

    def ss_fold(n, h):
        st = state[n]
        h0, h1 = (0, SSH) if h == 0 else (SSH, NCH)
        nc.gpsimd.tensor_tensor(
            out=st["t64"][:, h0:h1, :],
            in0=st["x2"][:, h0:h1, 0:64],
            in1=st["x2"][:, h0:h1, 64:128],
            op=ALU.add,
        )

    def ss_fold2(n, h):
        st = state[n]
        h0, h1 = (0, SSH) if h == 0 else (SSH, NCH)
        nc.gpsimd.tensor_tensor(
            out=st["t32"][:, h0:h1, :],
            in0=st["t64"][:, h0:h1, 0:32],
            in1=st["t64"][:, h0:h1, 32:64],
            op=ALU.add,
        )

    def ss_red(n, h):
        st = state[n]
        h0, h1 = (0, SSH) if h == 0 else (SSH, NCH)
        nc.vector.tensor_reduce(
            out=st["ss"][:, h0:h1], in_=st["t32"][:, h0:h1, :], axis=X_AX, op=ALU.add
        )

    def ss_finish(n, h):
        st = state[n]
        ss = st["ss"]
        if h == 0:
            st["lns"] = smalls.tile([128, NCH], f32, tag="lns", name="lns")
            st["is_"] = smalls.tile([128, NCH], f32, tag="is", name="is")
        lns, is_ = st["lns"], st["is_"]
        h0, h1 = (0, SSH) if h == 0 else (SSH, NCH)
        # inv_s = exp(-0.5*ln(ss)); Ln+Exp share one ACT table set
        nc.scalar.activation(lns[:, h0:h1], ss[:, h0:h1], AF.Ln)
        nc.scalar.activation(is_[:, h0:h1], lns[:, h0:h1], AF.Exp, scale=-0.5)
        # s-col: xtr[:, c, 128] = ss * inv_s = ||x_p||
        nc.gpsimd.tensor_tensor(
            out=st["xtr"][:, h0:h1, 128],
            in0=ss[:, h0:h1],
            in1=is_[:, h0:h1],
            op=ALU.mult,
        )

    # ---- col-tiled acc matmuls of sample m ----
    def emit_acc_chunks(m, c0, c1):
        st = state[m]
        xtr, sbt = st["xtr"], st["sbt"]
        if m not in cstate:
            cstate[m] = pp_acc.tile([128, 129], f32, tag="acc", name="acc")
        acc_p = cstate[m]
        for c in range(c0, min(c1, NCH)):
            p0, cw = CHUNKS[c]
            half = c % 2
            nc.tensor.matmul(
                acc_p[64 * half : 64 * half + 64, :],
                lhsT=sbt[:cw, c, :],
                rhs=xtr[:cw, c, 0:129],
                start=(c < 2),
                stop=(c >= NCH - 2),
                tile_position=(0, 64 * half),
            )

    agg_all = tails.tile([K, NS, D], f32)
    ssa_all = tails.tile([K, NS], f32)

    def finish_acc(m):
        acc_p = cstate.pop(m)
        # agg = even-chunk half + odd-chunk half; same for sum_sa column
        nc.scalar.copy(agg_all[:, m, :], acc_p[0:64, 0:D])
        nc.vector.tensor_tensor(
            out=agg_all[:, m, :],
            in0=agg_all[:, m, :],
            in1=acc_p[64:128, 0:D],
            op=ALU.add,
        )
        nc.scalar.copy(ssa_all[:, m : m + 1], acc_p[0:64, 128:129])
        nc.vector.tensor_tensor(
            out=ssa_all[:, m : m + 1],
            in0=ssa_all[:, m : m + 1],
            in1=acc_p[64:128, 128:129],
            op=ALU.add,
        )
        state.pop(m)

    def emit_tail(n0, n1):
        nn = n1 - n0
        agg_h = agg_all[:, n0:n1, :]
        ssa_h = ssa_all[:, n0:n1]
        vl = tails.tile([K, nn, D], f32, tag=f"t_vl{n0}", name="vl")
        vsq = tails.tile([K, nn, D], f32, tag=f"t_vsq{n0}", name="vsq")
        q = tails.tile([K, nn], f32, tag=f"t_q{n0}", name="q")
        qm = tails.tile([K, nn], f32, tag=f"t_qm{n0}", name="qm")
        lq = tails.tile([K, nn], f32, tag=f"t_lq{n0}", name="lq")
        isq = tails.tile([K, nn], f32, tag=f"t_isq{n0}", name="isq")
        isq2 = tails.tile([K, nn], f32, tag=f"t_isq2{n0}", name="isq2")
        u = tails.tile([K, nn], f32, tag=f"t_u{n0}", name="u")
        g = tails.tile([K, nn], f32, tag=f"t_g{n0}", name="g")
        gm = tails.tile([K, nn], f32, tag=f"t_gm{n0}", name="gm")
        lgm = tails.tile([K, nn], f32, tag=f"t_lgm{n0}", name="lgm")
        gis = tails.tile([K, nn], f32, tag=f"t_gis{n0}", name="gis")
        sall = tails.tile([K, nn], f32, tag=f"t_s{n0}", name="sall")
        vf = tails.tile([K, nn, D], f32, tag=f"t_vf{n0}", name="vf")

        # vl = agg - ssa * cent
        nc.gpsimd.tensor_tensor(
            out=vl[:], in0=bcast(ssa_h, D), in1=mid_bcast(cent_s[:], nn), op=ALU.mult
        )
        nc.vector.tensor_tensor(out=vl[:], in0=agg_h, in1=vl[:], op=ALU.subtract)
        # q = rowsum(vl^2) per (k, n)
        nc.scalar.activation(vsq[:], vl[:], AF.Square)
        nc.vector.tensor_reduce(out=q[:], in_=vsq[:], axis=X_AX, op=ALU.add)
        nc.vector.tensor_scalar_max(qm[:], q[:], 1e-24)
        nc.scalar.activation(lq[:], qm[:], AF.Ln)
        nc.scalar.activation(isq[:], lq[:], AF.Exp, scale=-0.5)
        # g[n] = sum_k q_k * isq_k^2, all-reduced across partitions
        nc.vector.tensor_tensor(out=isq2[:], in0=isq[:], in1=isq[:], op=ALU.mult)
        nc.vector.tensor_tensor(out=u[:], in0=q[:], in1=isq2[:], op=ALU.mult)
        nc.gpsimd.partition_all_reduce(
            g[:], u[:], channels=K, reduce_op=bass_isa.ReduceOp.add
        )
        nc.vector.tensor_scalar_max(gm[:], g[:], 1e-24)
        nc.scalar.activation(lgm[:], gm[:], AF.Ln)
        nc.scalar.activation(gis[:], lgm[:], AF.Exp, scale=-0.5)
        # s = isq * gis; vf = vl * s
        nc.vector.tensor_tensor(out=sall[:], in0=isq[:], in1=gis[:], op=ALU.mult)
        nc.gpsimd.tensor_tensor(
            out=vf[:], in0=vl[:], in1=bcast(sall[:], D), op=ALU.mult
        )
        nc.sync.dma_start(
            out=out_ap.rearrange("n k d -> k n d")[:, n0:n1, :], in_=vf[:]
        )

    # ---- main schedule ----
    state[0] = {"xs": emit_load(0)}
    acc_per_wave = (NCH + len(WAVES) - 1) // len(WAVES)

    def chain_round(p, wave=None, accp=None, lg=None):
        """Emit the ss-chain + softmax of sample p, striped with the current
        round's transpose waves (wave: callable emitting wave w), acc
        portions, and sample p's logits matmuls (lg: callable emitting lg
        tile t). Any of wave/accp/lg may be no-ops (drain)."""
        wave = wave or (lambda w: None)
        accp = accp or (lambda w: None)
        lg = lg or (lambda t: None)
        live = p is not None

        def c(f, *a):
            if live:
                f(p, *a)

        c(ss_alloc)
        c(ss_sq, 0)
        wave(0)
        c(lambda p: lg(0))
        c(ss_sq, 1)
        c(ss_fold, 0)
        accp(0)
        wave(1)
        c(lambda p: lg(1))
        c(ss_fold2, 0)
        c(ss_red, 0)
        c(ss_finish, 0)
        c(ss_fold, 1)
        accp(1)
        wave(2)
        c(lambda p: lg(2))
        c(sm_slg, [0, 1])
        c(ss_fold2, 1)
        c(ss_red, 1)
        c(ss_finish, 1)
        accp(2)
        wave(3)
        c(lambda p: lg(3))
        c(sm_slg, [2])
        c(sm_exp, 0)
        accp(3)
        wave(4)
        c(lambda p: lg(4))
        c(sm_slg, [3, 4])
        c(sm_zz, 0)
        c(sm_exp, 1)
        accp(4)
        c(sm_zz, 1)
        c(sm_tsc)
        if live:
            state[p].pop("lgp")

    def round_n(n):
        if n + 1 < NS:
            state[n + 1] = {"xs": emit_load(n + 1)}
        state[n]["xtr"] = xtrpool.tile(
            [128, NCH, XTS], bf16, tag="xtr", name="xtr"
        )
        if n >= PIPE + 1:
            finish_acc(n - PIPE - 1)
        if n >= 2:
            sm_sbt(n - 2)

        def accp(w):
            if n >= PIPE:
                emit_acc_chunks(n - PIPE, w * acc_per_wave, (w + 1) * acc_per_wave)

        chain_round(
            n - 1 if n >= 1 else None,
            wave=lambda w: emit_wave(n, w),
            accp=accp,
            lg=(lambda t: emit_lg_tile(n - 1, t)) if n >= 1 else None,
        )

    for n in range(NS):
        round_n(n)

    # drain pseudo-round: chain + logits + softmax of the last sample,
    # the acc matmuls of NS-PIPE, then remaining accs and the tail
    m = NS - 1
    finish_acc(NS - PIPE - 1)
    sm_sbt(NS - 2)

    def accp_d(w):
        emit_acc_chunks(NS - PIPE, w * acc_per_wave, (w + 1) * acc_per_wave)

    chain_round(m, accp=accp_d, lg=lambda t: emit_lg_tile(m, t))
    finish_acc(NS - PIPE)
    sm_sbt(m)
    for mm in range(NS - PIPE + 1, NS):
        emit_acc_chunks(mm, 0, NCH)
        finish_acc(mm)
    emit_tail(0, NS)


def kernel(x, conv_w, centroids):
    import ml_dtypes
    from concourse.bass_utils import run_bass_kernel_spmd

    if "nc" not in _CACHE:
        _CACHE["nc"] = _build_nc()
    nc = _CACHE["nc"]

    x = np.ascontiguousarray(
        np.asarray(x, dtype=np.float32).reshape(N, D, HW).astype(ml_dtypes.bfloat16)
    )
    wt = np.ascontiguousarray(
        np.asarray(conv_w, dtype=np.float32).T.astype(ml_dtypes.bfloat16)
    )
    cent = np.ascontiguousarray(np.asarray(centroids, dtype=np.float32))
    in_maps = [
        {"x": x[i * NS : (i + 1) * NS], "wt": wt, "cent": cent} for i in range(NCORES)
    ]
    res = run_bass_kernel_spmd(nc, in_maps, core_ids=list(range(NCORES))).results
    out = np.concatenate([r["out"].reshape(NS, K * D) for r in res], axis=0)
    return out


if __name__ == "__main__":
    rng = np.random.default_rng(0)
    xs = rng.standard_normal((N, D, 60, 80), dtype=np.float32)
    cw = (rng.standard_normal((K, D)) * 0.1).astype(np.float32)
    ct = rng.random((K, D), dtype=np.float32)
    o = kernel(x=xs, conv_w=cw, centroids=ct)
    print("kernel out", o.shape, o.dtype, np.abs(o).max())


# revision 24
# speedup vs baseline: 1.2232x; 1.2232x over previous
"""NetVLAD Trainium2 Bass kernel, v3 (bf16 matmuls, fused slg, col-tiled acc).

Full inputs in, full output out. Data-parallel over batch N=64 across 8
NeuronCores (8 samples per core); conv weight and centroids replicated.

Structure (per core, 8 samples, 38 pixel-chunks of 128 per sample):
  - x arrives bf16 (host-converted): half the HBM traffic, and every PE
    stream runs at 1 cycle/row (fp32 would be 4).
  - Per chunk: transpose-mode matmul writes x_c^T to a *bf16* PSUM bank
    (8 chunks/bank), evacuated to SBUF by DVE in the 2x packed mode /
    ACT; a second matmul with the same stationary x_c produces logits
    into an f32 PSUM pool large enough for a whole sample (5 banks).
  - slg = logits * inv_s is computed by DVE directly from logits PSUM
    (per-bank ops, step-0 broadcast of inv_s), which both evacuates and
    scales in one 1x pass and releases the logits banks.
  - ss = sum_d x^2: ACT squares, GPSIMD does one pairwise fold (128->64),
    DVE reduces the rest. inv_s = exp(-0.5 ln ss) on ACT.
  - softmax: exp on ACT, Z-reduce + reciprocal on DVE, t = inv_s/Z and
    sbt = et * t broadcasts on GPSIMD.
  - acc matmuls are column-tiled: even chunks accumulate into PSUM
    partitions 0:64, odd chunks into 64:128, running pairwise-concurrent
    in the PE array; the two halves are summed during evacuation.
  - Tail (VLAD normalizations) uses gpsimd.partition_all_reduce for the
    global norm, so no PSUM bank is needed for tiny matmuls.
  PSUM budget: 2 (xt bf16) + 5 (logits f32) + 1 (acc) = 8 banks.
"""

import sys

if "/opt/trn_rl_repo" not in sys.path:
    sys.path.insert(0, "/opt/trn_rl_repo")

import numpy as np
from contextlib import ExitStack

N, D, HW, K = 64, 128, 4800, 64
NCORES = 8
NS = N // NCORES  # samples per core

CHUNKS = [(i * 128, min(128, HW - i * 128)) for i in range((HW + 127) // 128)]
NCH = len(CHUNKS)  # 38: 37 full + one 64-wide
XTS = 130  # xtr row stride (129 used; 130 keeps 4B alignment of chunk rows)

# waves of chunks sharing one xt PSUM bank / one lg PSUM bank
WAVES = [(w * 8, min(8, NCH - w * 8)) for w in range((NCH + 7) // 8)]  # 5 waves
NLG = len(WAVES)

PIPE = 3  # acc matmuls of sample n-PIPE run during round n

_CACHE = {}


def _patch_act_tables():
    """Steer bacc's ACT table-set placement so ln/exp (and the cheap
    square/copy fillers) live in one set: a single ACT_TABLE_LOAD."""
    if _CACHE.get("act_patched"):
        return
    from concourse import bacc, mybir

    orig = bacc.get_activation_tables
    AF = mybir.ActivationFunctionType
    combo = "natural_log_exp_and_others"

    def patched(arch):
        t = {k: set(v) for k, v in orig(arch).items()}
        if combo in t:
            for name in t:
                if name != combo:
                    t[name] = t[name] - {AF.Ln, AF.Exp}
        return t

    bacc.get_activation_tables = patched
    _CACHE["act_patched"] = True


def _build_nc():
    import concourse.tile as tile
    from concourse import bacc, mybir

    _patch_act_tables()

    nc = bacc.Bacc(
        "TRN2",
        target_bir_lowering=False,
        debug=False,
        enable_asserts=False,
        num_devices=NCORES,
    )
    x_ap = nc.dram_tensor(
        "x", [NS, D, HW], mybir.dt.bfloat16, kind="ExternalInput"
    ).ap()
    wt_ap = nc.dram_tensor("wt", [D, K], mybir.dt.bfloat16, kind="ExternalInput").ap()
    cent_ap = nc.dram_tensor(
        "cent", [K, D], mybir.dt.float32, kind="ExternalInput"
    ).ap()
    out_ap = nc.dram_tensor(
        "out", [NS, K, D], mybir.dt.float32, kind="ExternalOutput"
    ).ap()

    with tile.TileContext(nc) as tc:
        with ExitStack() as ctx:
            _body(ctx, tc, out_ap, x_ap, wt_ap, cent_ap)
    nc.compile()
    return nc


def _body(ctx, tc, out_ap, x_ap, wt_ap, cent_ap):
    import concourse.bass as bass
    from concourse import bass_isa, masks, mybir

    nc = tc.nc
    f32 = mybir.dt.float32
    bf16 = mybir.dt.bfloat16
    AF = mybir.ActivationFunctionType
    ALU = mybir.AluOpType
    X_AX = mybir.AxisListType.X

    singles = ctx.enter_context(tc.tile_pool(name="singles", bufs=1))
    xspool = ctx.enter_context(tc.tile_pool(name="xspool", bufs=3))
    xtrpool = ctx.enter_context(tc.tile_pool(name="xtrpool", bufs=PIPE + 1))
    x2pool = ctx.enter_context(tc.tile_pool(name="x2pool", bufs=2))
    slgpool = ctx.enter_context(tc.tile_pool(name="slgpool", bufs=2))
    etpool = ctx.enter_context(tc.tile_pool(name="etpool", bufs=2))
    sbtpool = ctx.enter_context(tc.tile_pool(name="sbtpool", bufs=PIPE))
    smalls = ctx.enter_context(tc.tile_pool(name="smalls", bufs=2))
    tails = ctx.enter_context(tc.tile_pool(name="tails", bufs=1))
    pp_xt = ctx.enter_context(tc.tile_pool(name="pp_xt", bufs=2, space="PSUM"))
    pp_lg = ctx.enter_context(tc.tile_pool(name="pp_lg", bufs=NLG, space="PSUM"))
    pp_acc = ctx.enter_context(tc.tile_pool(name="pp_acc", bufs=1, space="PSUM"))

    def bcast(ap, n):
        # append a step-0 free dim: [..., n] broadcast view
        return bass.AP(tensor=ap.tensor, offset=ap.offset, ap=list(ap.ap) + [[0, n]])

    def mid_bcast(ap, n):
        # [p, f] -> [p, n, f] with step-0 middle dim
        return bass.AP(
            tensor=ap.tensor,
            offset=ap.offset,
            ap=[ap.ap[0], [0, n]] + list(ap.ap[1:]),
        )

    # constants
    identb = singles.tile([128, 128], bf16)
    masks.make_identity(nc, identb[:])
    wtb = singles.tile([D, K], bf16)
    nc.sync.dma_start(out=wtb[:], in_=wt_ap[:])
    cent_s = singles.tile([K, D], f32)
    nc.sync.dma_start(out=cent_s[:], in_=cent_ap[:])

    state = {}  # n -> dict of live tiles
    cstate = {}  # n -> acc psum tile

    def emit_load(n):
        xs = xspool.tile([D, HW], bf16, tag="xs", name="xs")
        nc.sync.dma_start(out=xs[:, 0 : HW // 2], in_=x_ap[n, :, 0 : HW // 2])
        nc.sync.dma_start(out=xs[:, HW // 2 :], in_=x_ap[n, :, HW // 2 :])
        return xs

    # ---- softmax part A of sample m, split into pieces interleaved with
    # the next round's waves (slg / exp / zz / recip / tsc) ----
    def sm_slg(m, tiles):
        st = state[m]
        is_ = st["is_"]
        if "slg" not in st:
            st["slg"] = slgpool.tile([128, NCH, K], bf16, tag="slg", name="slg")
            st["et"] = etpool.tile([128, NCH, K], bf16, tag="et", name="et")
            st["zz"] = smalls.tile([128, NCH], f32, tag="zz", name="zz")
        slg = st["slg"]
        # slg = lg * inv_s straight out of PSUM (evac + scale in one pass);
        # releases lg bank t for the next round's logits matmuls
        for t in tiles:
            lg_p = st["lgp"][t]
            c0, wn = WAVES[t]
            nc.vector.tensor_tensor(
                out=slg[:, c0 : c0 + wn, :],
                in0=lg_p[:, 0:wn, :],
                in1=bcast(is_[:, c0 : c0 + wn], K),
                op=ALU.mult,
            )

    SMH = 2 * (NCH // 4)  # 18

    def sm_exp(m, h):
        st = state[m]
        h0, h1 = (0, SMH) if h == 0 else (SMH, NCH)
        nc.scalar.activation(
            st["et"][:, h0:h1, :], st["slg"][:, h0:h1, :], AF.Exp
        )

    def sm_zz(m, h):
        st = state[m]
        h0, h1 = (0, SMH) if h == 0 else (SMH, NCH)
        nc.vector.tensor_reduce(
            out=st["zz"][:, h0:h1], in_=st["et"][:, h0:h1, :], axis=X_AX, op=ALU.add
        )

    def sm_tsc(m):
        st = state[m]
        rr = smalls.tile([128, NCH], f32, tag="rr", name="rr")
        tsc = smalls.tile([128, NCH], bf16, tag="tsc", name="tsc")
        st["tsc"] = tsc
        nc.vector.reciprocal(rr[:], st["zz"][:])
        nc.vector.tensor_tensor(out=tsc[:], in0=st["is_"], in1=rr[:], op=ALU.mult)

    # ---- softmax part B of sample m (runs at round m+2 start: ready work
    # for GPSIMD while everything else in the round is still blocked) ----
    def sm_sbt(m):
        st = state[m]
        et, tsc = st["et"], st["tsc"]
        sbt = sbtpool.tile([128, NCH, K], bf16, tag="sbt", name="sbt")
        st["sbt"] = sbt
        for h0, h1 in ((0, SMH), (SMH, NCH)):
            nc.gpsimd.tensor_tensor(
                out=sbt[:, h0:h1, :],
                in0=et[:, h0:h1, :],
                in1=bcast(tsc[:, h0:h1], K),
                op=ALU.mult,
            )

    # ---- transpose waves of sample n ----
    def emit_wave(n, w):
        st = state[n]
        xs, xtr = st["xs"], st["xtr"]
        c0, wn = WAVES[w]
        xt_p = pp_xt.tile([128, 8, 128], bf16, tag="xt", name="xt")
        for j in range(wn):
            p0, cw = CHUNKS[c0 + j]
            nc.tensor.transpose(xt_p[:cw, j, :], xs[:, p0 : p0 + cw], identb[:])
        dst = xtr[:, c0 : c0 + wn, 0:128]
        src = xt_p[:, 0:wn, :]
        if w % 2 == 0:
            nc.vector.tensor_copy(dst, src)
        else:
            nc.scalar.copy(dst, src)

    def emit_lg_tile(n, t):
        st = state[n]
        xs = st["xs"]
        if "lgp" not in st:
            st["lgp"] = {}
        c0, wn = WAVES[t]
        lg_p = pp_lg.tile([128, 8, K], f32, tag="lg", name="lg")
        for j in range(wn):
            p0, cw = CHUNKS[c0 + j]
            nc.tensor.matmul(
                lg_p[:cw, j, :],
                lhsT=xs[:, p0 : p0 + cw],
                rhs=wtb[:],
                start=True,
                stop=True,
            )
        st["lgp"][t] = lg_p

    # ---- ss chain of sample n (round tail): is_ ready for round n+1 ----
    SSH = NCH // 2  # 19

    def ss_alloc(n):
        st = state[n]
        st["x2"] = x2pool.tile([128, NCH, 128], bf16, tag="x2", name="x2")
        st["t64"] = x2pool.tile([128, NCH, 64], bf16, tag="t64", name="t64")
        st["ss"] = smalls.tile([128, NCH], f32, tag="ss", name="ss")

    def ss_sq(n, h):
        st = state[n]
        h0, h1 = (0, SSH) if h == 0 else (SSH, NCH)
        nc.scalar.activation(
            st["x2"][:, h0:h1, :], st["xtr"][:, h0:h1, 0:128], AF.Square
        )

    def ss_fold(n, h):
        st = state[n]
        h0, h1 = (0, SSH) if h == 0 else (SSH, NCH)
        nc.gpsimd.tensor_tensor(
            out=st["t64"][:, h0:h1, :],
            in0=st["x2"][:, h0:h1, 0:64],
            in1=st["x2"][:, h0:h1, 64:128],
            op=ALU.add,
        )

    def ss_red(n, h):
        st = state[n]
        h0, h1 = (0, SSH) if h == 0 else (SSH, NCH)
        nc.vector.tensor_reduce(
            out=st["ss"][:, h0:h1], in_=st["t64"][:, h0:h1, :], axis=X_AX, op=ALU.add
        )

    def ss_finish(n, h):
        st = state[n]
        ss = st["ss"]
        if h == 0:
            st["lns"] = smalls.tile([128, NCH], f32, tag="lns", name="lns")
            st["is_"] = smalls.tile([128, NCH], f32, tag="is", name="is")
        lns, is_ = st["lns"], st["is_"]
        h0, h1 = (0, SSH) if h == 0 else (SSH, NCH)
        # inv_s = exp(-0.5*ln(ss)); Ln+Exp share one ACT table set
        nc.scalar.activation(lns[:, h0:h1], ss[:, h0:h1], AF.Ln)
        nc.scalar.activation(is_[:, h0:h1], lns[:, h0:h1], AF.Exp, scale=-0.5)
        # s-col: xtr[:, c, 128] = ss * inv_s = ||x_p||
        nc.vector.tensor_tensor(
            out=st["xtr"][:, h0:h1, 128],
            in0=ss[:, h0:h1],
            in1=is_[:, h0:h1],
            op=ALU.mult,
        )

    # ---- col-tiled acc matmuls of sample m ----
    def emit_acc_chunks(m, c0, c1):
        st = state[m]
        xtr, sbt = st["xtr"], st["sbt"]
        if m not in cstate:
            cstate[m] = pp_acc.tile([128, 129], f32, tag="acc", name="acc")
        acc_p = cstate[m]
        for c in range(c0, min(c1, NCH)):
            p0, cw = CHUNKS[c]
            half = c % 2
            nc.tensor.matmul(
                acc_p[64 * half : 64 * half + 64, :],
                lhsT=sbt[:cw, c, :],
                rhs=xtr[:cw, c, 0:129],
                start=(c < 2),
                stop=(c >= NCH - 2),
                tile_position=(0, 64 * half),
            )

    agg_all = tails.tile([K, NS, D], f32)
    ssa_all = tails.tile([K, NS], f32)

    def finish_acc(m):
        acc_p = cstate.pop(m)
        # agg = even-chunk half + odd-chunk half; same for sum_sa column
        nc.scalar.copy(agg_all[:, m, :], acc_p[0:64, 0:D])
        nc.vector.tensor_tensor(
            out=agg_all[:, m, :],
            in0=agg_all[:, m, :],
            in1=acc_p[64:128, 0:D],
            op=ALU.add,
        )
        nc.scalar.copy(ssa_all[:, m : m + 1], acc_p[0:64, 128:129])
        nc.vector.tensor_tensor(
            out=ssa_all[:, m : m + 1],
            in0=ssa_all[:, m : m + 1],
            in1=acc_p[64:128, 128:129],
            op=ALU.add,
        )
        state.pop(m)

    def emit_tail(n0, n1):
        nn = n1 - n0
        agg_h = agg_all[:, n0:n1, :]
        ssa_h = ssa_all[:, n0:n1]
        vl = tails.tile([K, nn, D], f32, tag=f"t_vl{n0}", name="vl")
        vsq = tails.tile([K, nn, D], f32, tag=f"t_vsq{n0}", name="vsq")
        q = tails.tile([K, nn], f32, tag=f"t_q{n0}", name="q")
        qm = tails.tile([K, nn], f32, tag=f"t_qm{n0}", name="qm")
        lq = tails.tile([K, nn], f32, tag=f"t_lq{n0}", name="lq")
        isq = tails.tile([K, nn], f32, tag=f"t_isq{n0}", name="isq")
        isq2 = tails.tile([K, nn], f32, tag=f"t_isq2{n0}", name="isq2")
        u = tails.tile([K, nn], f32, tag=f"t_u{n0}", name="u")
        g = tails.tile([K, nn], f32, tag=f"t_g{n0}", name="g")
        gm = tails.tile([K, nn], f32, tag=f"t_gm{n0}", name="gm")
        lgm = tails.tile([K, nn], f32, tag=f"t_lgm{n0}", name="lgm")
        gis = tails.tile([K, nn], f32, tag=f"t_gis{n0}", name="gis")
        sall = tails.tile([K, nn], f32, tag=f"t_s{n0}", name="sall")
        vf = tails.tile([K, nn, D], f32, tag=f"t_vf{n0}", name="vf")

        # vl = agg - ssa * cent
        nc.gpsimd.tensor_tensor(
            out=vl[:], in0=bcast(ssa_h, D), in1=mid_bcast(cent_s[:], nn), op=ALU.mult
        )
        nc.vector.tensor_tensor(out=vl[:], in0=agg_h, in1=vl[:], op=ALU.subtract)
        # q = rowsum(vl^2) per (k, n)
        nc.scalar.activation(vsq[:], vl[:], AF.Square)
        nc.vector.tensor_reduce(out=q[:], in_=vsq[:], axis=X_AX, op=ALU.add)
        nc.vector.tensor_scalar_max(qm[:], q[:], 1e-24)
        nc.scalar.activation(lq[:], qm[:], AF.Ln)
        nc.scalar.activation(isq[:], lq[:], AF.Exp, scale=-0.5)
        # g[n] = sum_k q_k * isq_k^2, all-reduced across partitions
        nc.vector.tensor_tensor(out=isq2[:], in0=isq[:], in1=isq[:], op=ALU.mult)
        nc.vector.tensor_tensor(out=u[:], in0=q[:], in1=isq2[:], op=ALU.mult)
        nc.gpsimd.partition_all_reduce(
            g[:], u[:], channels=K, reduce_op=bass_isa.ReduceOp.add
        )
        nc.vector.tensor_scalar_max(gm[:], g[:], 1e-24)
        nc.scalar.activation(lgm[:], gm[:], AF.Ln)
        nc.scalar.activation(gis[:], lgm[:], AF.Exp, scale=-0.5)
        # s = isq * gis; vf = vl * s
        nc.vector.tensor_tensor(out=sall[:], in0=isq[:], in1=gis[:], op=ALU.mult)
        nc.gpsimd.tensor_tensor(
            out=vf[:], in0=vl[:], in1=bcast(sall[:], D), op=ALU.mult
        )
        nc.sync.dma_start(
            out=out_ap.rearrange("n k d -> k n d")[:, n0:n1, :], in_=vf[:]
        )

    # ---- main schedule ----
    state[0] = {"xs": emit_load(0)}
    acc_per_wave = (NCH + len(WAVES) - 1) // len(WAVES)

    def chain_round(p, wave=None, accp=None, lg=None):
        """Emit the ss-chain + softmax of sample p, striped with the current
        round's transpose waves (wave: callable emitting wave w), acc
        portions, and sample p's logits matmuls (lg: callable emitting lg
        tile t). Any of wave/accp/lg may be no-ops (drain)."""
        wave = wave or (lambda w: None)
        accp = accp or (lambda w: None)
        lg = lg or (lambda t: None)
        live = p is not None

        def c(f, *a):
            if live:
                f(p, *a)

        c(ss_alloc)
        c(ss_sq, 0)
        wave(0)
        c(lambda p: lg(0))
        c(ss_sq, 1)
        c(ss_fold, 0)
        accp(0)
        wave(1)
        c(lambda p: lg(1))
        c(ss_red, 0)
        c(ss_finish, 0)
        c(ss_fold, 1)
        accp(1)
        wave(2)
        c(lambda p: lg(2))
        c(sm_slg, [0, 1])
        c(ss_red, 1)
        c(ss_finish, 1)
        accp(2)
        wave(3)
        c(lambda p: lg(3))
        c(sm_slg, [2])
        c(sm_exp, 0)
        accp(3)
        wave(4)
        c(lambda p: lg(4))
        c(sm_slg, [3, 4])
        c(sm_zz, 0)
        c(sm_exp, 1)
        accp(4)
        c(sm_zz, 1)
        c(sm_tsc)
        if live:
            state[p].pop("lgp")

    def round_n(n):
        if n + 1 < NS:
            state[n + 1] = {"xs": emit_load(n + 1)}
        state[n]["xtr"] = xtrpool.tile(
            [128, NCH, XTS], bf16, tag="xtr", name="xtr"
        )
        if n >= PIPE + 1:
            finish_acc(n - PIPE - 1)
        if n >= 2:
            sm_sbt(n - 2)

        def accp(w):
            if n >= PIPE:
                emit_acc_chunks(n - PIPE, w * acc_per_wave, (w + 1) * acc_per_wave)

        chain_round(
            n - 1 if n >= 1 else None,
            wave=lambda w: emit_wave(n, w),
            accp=accp,
            lg=(lambda t: emit_lg_tile(n - 1, t)) if n >= 1 else None,
        )

    for n in range(NS):
        round_n(n)

    # drain pseudo-round: chain + logits + softmax of the last sample,
    # the acc matmuls of NS-PIPE, then remaining accs and the tail
    m = NS - 1
    finish_acc(NS - PIPE - 1)
    sm_sbt(NS - 2)

    def accp_d(w):
        emit_acc_chunks(NS - PIPE, w * acc_per_wave, (w + 1) * acc_per_wave)

    chain_round(m, accp=accp_d, lg=lambda t: emit_lg_tile(m, t))
    finish_acc(NS - PIPE)
    sm_sbt(m)
    for mm in range(NS - PIPE + 1, NS):
        emit_acc_chunks(mm, 0, NCH)
        finish_acc(mm)
    emit_tail(0, NS)


def kernel(x, conv_w, centroids):
    import ml_dtypes
    from concourse.bass_utils import run_bass_kernel_spmd

    if "nc" not in _CACHE:
        _CACHE["nc"] = _build_nc()
    nc = _CACHE["nc"]

    x = np.ascontiguousarray(
        np.asarray(x, dtype=np.float32).reshape(N, D, HW).astype(ml_dtypes.bfloat16)
    )
    wt = np.ascontiguousarray(
        np.asarray(conv_w, dtype=np.float32).T.astype(ml_dtypes.bfloat16)
    )
    cent = np.ascontiguousarray(np.asarray(centroids, dtype=np.float32))
    in_maps = [
        {"x": x[i * NS : (i + 1) * NS], "wt": wt, "cent": cent} for i in range(NCORES)
    ]
    res = run_bass_kernel_spmd(nc, in_maps, core_ids=list(range(NCORES))).results
    out = np.concatenate([r["out"].reshape(NS, K * D) for r in res], axis=0)
    return out


if __name__ == "__main__":
    rng = np.random.default_rng(0)
    xs = rng.standard_normal((N, D, 60, 80), dtype=np.float32)
    cw = (rng.standard_normal((K, D)) * 0.1).astype(np.float32)
    ct = rng.random((K, D), dtype=np.float32)
    o = kernel(x=xs, conv_w=cw, centroids=ct)
    print("kernel out", o.shape, o.dtype, np.abs(o).max())
